# revision 76
# baseline (speedup 1.0000x reference)
"""Trainium2 Bass kernel for ContextualLanguageRefinement (sparse local attention).

Math (per batch b):
  Q = h @ W_Q / sqrt(DS); K = h @ W_K
  scores[t,s] = Q[t].K[s], banded |t-s|<=3, softmax over s
  out = softmax((attn @ h @ W_proj) / tau)  computed as  attn @ (h @ W_proj)

Sharding: data-parallel over batch B=8 across the 8 NeuronCores; the small
weights are replicated (concatenated + pre-scaled + bf16-cast on host).
h is transposed + bf16-cast on host so the device reads h^T directly
(d on partitions) with plain strided DMAs -- no on-device transposes.

Per-core device pipeline:
  1. Weights arrive in three pieces (K cols, Q cols, hp cols) interleaved
     with h^T column-chunk loads, ordered so group 0's K matmuls start as
     soon as the first chunk lands; a short burst of dummy "warm-up"
     matmuls keeps the PE busy through the DMA wait so the p-state ramp
     reaches full clock before real work.
  2. Group 0's projection streams chunk-by-chunk (K immediately, Q lagging
     three chunks) against 4 concurrent psum banks; groups 1-3 use dense
     per-m passes.  Fused projection YT = Wcat^T @ hT gives Q^T, K^T
     (padded, 8-col halos between groups) and hp^T per 512-token group.
  3. hp^T is re-transposed once into 17 global 128-token chunks hptT
     [128, 17, 33] (col 32 = ones), chunk i holding hp[s = 128i - 8 + p]
     -- exactly the score-window row alignment, so per-block combines need
     no per-block transposes.
  4. Per 128-token block: banded scores S^T over a 144-wide s-window into
     one psum bank, mask+exp, one PE matmul pair against hptT chunks gives
     [attn-weighted hp | band denominator]; the raw 33-vector per token is
     stored and the final cheap 32-way softmax (divide by denominator,
     exponentiate, normalize) runs on host.  Blocks are woven between
     projection passes so mask/exp latencies hide behind matmuls; the
     last eight blocks weave against all five of group 3's passes.
"""

import numpy as np
import ml_dtypes

import concourse.bass as bass
import concourse.bacc as bacc
import concourse.tile as tile
from concourse import mybir
from concourse.bass_utils import run_bass_kernel_spmd

F32 = mybir.dt.float32
BF16 = mybir.dt.bfloat16

B, T, D = 8, 2048, 1024
DS, KL = 256, 32
SCALE = float(np.sqrt(DS))
MW = 2 * DS + KL          # 544 concatenated projection cols
NJ = T // 512             # 4 column groups
NB = T // 128             # 16 token blocks
NEG = -1e9

N_CORES = 8


def build_nc():
    nc = bacc.Bacc("TRN2", target_bir_lowering=False, debug=False,
                   dynamic_dma_scratch_size=131072)

    ht_d = nc.dram_tensor("ht", [D, T], BF16, kind="ExternalInput")
    w_d = nc.dram_tensor("wcat", [D, MW], BF16, kind="ExternalInput")
    o_d = nc.dram_tensor("out", [T, KL + 1], F32, kind="ExternalOutput")

    with tile.TileContext(nc) as tc:
        with (
            tc.tile_pool(name="persist", bufs=1) as pp,
            tc.tile_pool(name="blk", bufs=16) as bp,
            tc.tile_pool(name="stgp", bufs=16) as sg,
            tc.tile_pool(name="ppsum", bufs=4, space="PSUM") as ppsum,
            tc.tile_pool(name="tpsum", bufs=1, space="PSUM") as tpsum,
            tc.tile_pool(name="bpsum", bufs=3, space="PSUM") as bpsum,
        ):
            # ---------------- persistent tiles ----------------
            wc = pp.tile([128, 8, MW], BF16, tag="wc")
            # hbt[p, c, t] = h[t, 128c + p]
            hbt = pp.tile([128, 8, T], BF16, tag="hbt")
            # Q^T per j: [128, dsc, 512]; K^T padded per j: cols = s-window
            # col g <-> s = 512j - 8 + g; g in [0,640) (zeros outside [8,520))
            qt = [pp.tile([128, 2, 512], BF16, tag=f"qt{j}", name=f"qt{j}") for j in range(NJ)]
            ktp = [pp.tile([128, 2, 640], BF16, tag=f"ktp{j}", name=f"ktp{j}") for j in range(NJ)]
            hpt = [pp.tile([32, 528], BF16, tag=f"hpt{j}", name=f"hpt{j}") for j in range(NJ)]
            # hp^T re-chunked: chunk i row p = hp[s = 128i - 8 + p]; col 32 = 1
            hptT = pp.tile([128, 17, KL + 1], BF16, tag="hptT")
            mask = pp.tile([128, 256], BF16, tag="mask")
            mask0 = pp.tile([128, 256], BF16, tag="mask0")
            mask15 = pp.tile([128, 256], BF16, tag="mask15")
            ident = pp.tile([32, 32], BF16, tag="ident")
            wz = pp.tile([128, 512], BF16, tag="wz")

            def warmup(n):
                # keep the PE continuously busy through the initial DMA wait
                # so the p-state ramp reaches full clock before real matmuls
                nc.gpsimd.memset(wz[:], 0.0)
                for k in range(n):
                    # share the projection psum slots (tag "pps"); the warm
                    # tiles have no readers, so rotation costs nothing
                    ps = ppsum.tile([128, 512], F32, tag="pps",
                                    name=f"warm{k}")
                    nc.tensor.matmul(ps[:], wz[:, 0:128], wz[:],
                                     start=True, stop=True)

            # ---------------- DMA loads (HWDGE via SP) ----------------
            def load_wc(m0, m1):
                # wc[p, c, m0:m1] = wcat[128c + p, m0:m1]
                nc.sync.dma_start(out=wc[:, :, m0:m1], in_=bass.AP(
                    tensor=w_d[:].tensor, offset=m0,
                    ap=[[MW, 128], [128 * MW, 8], [1, m1 - m0]]))

            def load_ht_j0_chunk(c, nc_chunks=1):
                # hbt[:, c:c+n, 0:512] <- ht[128c:128(c+n), 0:512]
                nc.sync.dma_start(
                    out=hbt[:, c:c + nc_chunks, 0:512],
                    in_=bass.AP(tensor=ht_d[:].tensor, offset=128 * c * T,
                                ap=[[T, 128], [128 * T, nc_chunks], [1, 512]]))

            def load_ht_group(j):
                # hbt[:, c, 512j:512j+512] <- ht[128c+p, 512j + t] for all c
                nc.sync.dma_start(
                    out=hbt[:, :, 512 * j:512 * j + 512],
                    in_=bass.AP(tensor=ht_d[:].tensor, offset=512 * j,
                                ap=[[T, 128], [128 * T, 8], [1, 512]]))

            # masks over the packed score sheet [p, 0:128]=window A, [p,128:256]=B
            # A: s = t0-8+p, t = t0+f  -> band iff p-f in [5,11]
            # B: s = t0+120+p, t = t0+(g-128) -> band iff g-p in [245,251]
            def build_masks():
              # multiplicative 0/1 band masks (bf16): applied to exp(scores)
              for mk in (mask, mask0, mask15):
                nc.gpsimd.memset(mk[:], 1.0)
                nc.gpsimd.affine_select(
                    out=mk[:, 0:128], in_=mk[:, 0:128],
                    compare_op=mybir.AluOpType.is_ge, fill=0.0,
                    base=-5, channel_multiplier=1, pattern=[[-1, 128]])
                nc.gpsimd.affine_select(
                    out=mk[:, 0:128], in_=mk[:, 0:128],
                    compare_op=mybir.AluOpType.is_ge, fill=0.0,
                    base=11, channel_multiplier=-1, pattern=[[1, 128]])
                nc.gpsimd.affine_select(
                    out=mk[:, 128:256], in_=mk[:, 128:256],
                    compare_op=mybir.AluOpType.is_ge, fill=0.0,
                    base=128 - 245, channel_multiplier=-1, pattern=[[1, 128]])
                nc.gpsimd.affine_select(
                    out=mk[:, 128:256], in_=mk[:, 128:256],
                    compare_op=mybir.AluOpType.is_ge, fill=0.0,
                    base=251 - 128, channel_multiplier=1, pattern=[[-1, 128]])
              # block 0 extra: window-A rows p<8 are s<0 -> invalid
              nc.gpsimd.affine_select(
                  out=mask0[:, 0:128], in_=mask0[:, 0:128],
                  compare_op=mybir.AluOpType.is_ge, fill=0.0,
                  base=-8, channel_multiplier=1, pattern=[[0, 128]])
              # block 15 extra: window-B rows p>7 are s>=2048 -> invalid
              nc.gpsimd.affine_select(
                  out=mask15[:, 128:256], in_=mask15[:, 128:256],
                  compare_op=mybir.AluOpType.is_ge, fill=0.0,
                  base=7, channel_multiplier=-1, pattern=[[0, 128]])

              # identity (bf16) for PE transposes of hp^T windows
              nc.gpsimd.memset(ident[:], 0.0)
              nc.gpsimd.affine_select(
                  out=ident[:], in_=ident[:], compare_op=mybir.AluOpType.not_equal,
                  fill=1.0, base=0, channel_multiplier=1, pattern=[[-1, 32]])

            # zero the K^T / hp^T padding columns.  [520:640) of every group
            # also guards the window-B matmul's over-read (masked after exp,
            # but must be finite); ones column of hptT.
            def build_pads():
                for j in range(NJ):
                    nc.vector.memset(ktp[j][:, :, 520:640], 0.0)
                    nc.vector.memset(hpt[j][:, 520:528], 0.0)
                nc.vector.memset(ktp[0][:, :, 0:8], 0.0)   # s < 0
                nc.vector.memset(hpt[0][:, 0:8], 0.0)
                nc.vector.memset(hptT[:, :, KL:KL + 1], 1.0)
                # chunk 16 rows 16:128 are never read by combines, but zero
                # them so the tile has no uninitialized reads under the sim.
                # (build_sliver(16) later overwrites rows 0:16.)
                nc.vector.memset(hptT[:, 16, 0:KL], 0.0)

            # -------- projection evacuation: psum -> qt/ktp/hpt (+halos) ----
            def evac(j, m, ps):
                if m in (3, 4):
                    cp = lambda o, i: nc.scalar.copy(out=o, in_=i)
                else:
                    cp = lambda o, i: nc.vector.tensor_copy(out=o, in_=i)
                if m < 2:
                    cp(qt[j][:, m, :], ps[:])
                elif m < 4:
                    cp(ktp[j][:, m - 2, 8:520], ps[:])
                    if j > 0:  # tail overlap of previous group (s>=512j)
                        cp(ktp[j - 1][:, m - 2, 520:528], ps[:, 0:8])
                    if j < NJ - 1:  # head of next group
                        cp(ktp[j + 1][:, m - 2, 0:8], ps[:, 504:512])
                else:
                    cp(hpt[j][:, 8:520], ps[:, 0:512])
                    if j > 0:
                        cp(hpt[j - 1][:, 520:528], ps[0:32, 0:8])
                    if j < NJ - 1:
                        cp(hpt[j + 1][:, 0:8], ps[0:32, 504:512])

            MOFF = {0: 0, 1: 128, 2: 256, 3: 384, 4: 512}

            def pass_m(j, m, t0=0, t1=512):
                # one projection m-pass over all 8 d-chunks (m-outer, c-inner)
                moff = MOFF[m]
                mp = 32 if m == 4 else 128
                ps = ppsum.tile([mp, 512], F32, tag="pps")
                for c in range(8):
                    nc.tensor.matmul(
                        ps[:, 0:t1 - t0], wc[:, c, moff:moff + mp],
                        hbt[:, c, 512 * j + t0:512 * j + t1],
                        start=(c == 0), stop=(c == 7))
                if (t0, t1) == (0, 512):
                    evac(j, m, ps)
                else:
                    assert m < 2
                    nc.scalar.copy(out=qt[j][:, m, t0:t1],
                                   in_=ps[:, 0:t1 - t0])

            def pass_j0_streamed():
                # group 0: consume hbt chunks as DMAs land.  One c-outer
                # sweep over 4 psum banks keeps per-chunk PE work (852ns)
                # above the DMA arrival spacing so the PE never idles (and
                # never drops out of full p-state); hp runs dense after.
                ps = {m: ppsum.tile([128, 512], F32, tag="pps",
                                    name=f"j0ps{m}")
                      for m in (2, 3, 0, 1)}

                def mm(m, c):
                    nc.tensor.matmul(
                        ps[m][:], wc[:, c, MOFF[m]:MOFF[m] + 128],
                        hbt[:, c, 0:512],
                        start=(c == 0), stop=(c == 7))

                # K first (its wc half arrives first); Q lags three chunks
                for c in range(8):
                    mm(2, c)
                    mm(3, c)
                    if c >= 3:
                        mm(0, c - 3)
                        mm(1, c - 3)
                for c in (5, 6, 7):
                    mm(0, c)
                    mm(1, c)
                # evacuate in DVE/ACT halves so the psum slots free quickly
                # (group 1's first pass reuses them)
                for m in (2, 3, 0, 1):
                    if m < 2:
                        nc.vector.tensor_copy(out=qt[0][:, m, 0:256],
                                              in_=ps[m][:, 0:256])
                        nc.scalar.copy(out=qt[0][:, m, 256:512],
                                       in_=ps[m][:, 256:512])
                    else:
                        nc.vector.tensor_copy(out=ktp[0][:, m - 2, 8:264],
                                              in_=ps[m][:, 0:256])
                        nc.scalar.copy(out=ktp[0][:, m - 2, 264:520],
                                       in_=ps[m][:, 256:512])
                        nc.scalar.copy(out=ktp[1][:, m - 2, 0:8],
                                       in_=ps[m][:, 504:512])
                pass_m(0, 4)

            # -------- hptT chunk builders (PE transpose + evac copy) --------
            def build_chunk(i):
                # full chunk i from hpt[j], cols [128r, 128r+128), j=i//4
                j, r = i // 4, i % 4
                pt = tpsum.tile([128, KL], BF16, tag="tp")
                nc.tensor.transpose(pt[:], hpt[j][:, 128 * r:128 * r + 128],
                                    ident[:])
                nc.vector.tensor_copy(out=hptT[:, i, 0:KL], in_=pt[:])

            def build_sliver(i):
                # rows 0:16 of chunk i = 4j+4 from hpt[j] cols [512, 528)
                j = i // 4 - 1
                pt = tpsum.tile([16, KL], BF16, tag="tp")
                nc.tensor.transpose(pt[:], hpt[j][:, 512:528], ident[:])
                nc.vector.tensor_copy(out=hptT[0:16, i, 0:KL], in_=pt[:])

            # ---------------- per-block attention ----------------
            # split into a scores phase (PE scores + mask + exp) and a
            # combine phase (PE combine + evac + store) so projection
            # passes can slot between them and hide the DVE/ACT latency
            blk_state = {}

            def block_scores(i):
                j, r = i // 4, i % 4
                g0 = 128 * r

                # one psum bank per block: cols 0:256 banded scores S^T
                # (window A in 0:128, B in 128:256), cols 256:289 combine
                bps = bpsum.tile([128, 512], F32, tag="bps")
                for dsc in range(2):
                    nc.tensor.matmul(
                        bps[:, 0:128], ktp[j][:, dsc, g0:g0 + 128],
                        qt[j][:, dsc, g0:g0 + 128],
                        start=(dsc == 0), stop=(dsc == 1))
                for dsc in range(2):
                    nc.tensor.matmul(
                        bps[:, 128:256], ktp[j][:, dsc, g0 + 128:g0 + 256],
                        qt[j][:, dsc, g0:g0 + 128],
                        start=(dsc == 0), stop=(dsc == 1))

                msk = mask0 if i == 0 else (mask15 if i == NB - 1 else mask)
                # exp the raw scores straight from psum (band values are
                # O(1), junk entries stay finite), then zero out-of-band
                # with a bf16 0/1 mask multiply -- all-SBUF 2-byte operands
                # hit the DVE fast path and one pipeline stage disappears
                ee = bp.tile([128, 256], BF16, tag="ee")
                nc.scalar.activation(out=ee[:], in_=bps[:, 0:256],
                                     func=mybir.ActivationFunctionType.Exp)
                est = bp.tile([128, 256], BF16, tag="est")
                nc.vector.tensor_mul(out=est[:], in0=ee[:], in1=msk[:])
                blk_state[i] = (bps, est)

            def block_combine(i):
                bps, est = blk_state.pop(i)
                # combine: [128t, 33] = sum_s EST[s, t] * [hp | 1][s]
                lp0 = 256
                nc.tensor.matmul(bps[:, lp0:lp0 + KL + 1], est[:, 0:128],
                                 hptT[:, i, :], start=True, stop=False)
                nc.tensor.matmul(bps[:, lp0:lp0 + KL + 1],
                                 est[0:16, 128:256], hptT[0:16, i + 1, :],
                                 start=False, stop=True)
                # evacuate [combine | denom] to SBUF (frees the psum bank)
                # and store raw; the cheap 32-way softmax finishes on host
                lpc = sg.tile([128, KL + 1], F32, tag="stg")
                nc.vector.tensor_copy(out=lpc[:], in_=bps[:, lp0:lp0 + KL + 1])
                nc.sync.dma_start(out=o_d[128 * i:128 * (i + 1), :],
                                  in_=lpc[:])

            # ---------------- emission order ----------------
            load_wc(256, 512)          # K cols first
            for c in range(3):
                load_ht_j0_chunk(c)
            load_wc(0, 256)            # Q cols (needed from the c=3 backfill)
            for c in range(3, 8):
                load_ht_j0_chunk(c)
            load_wc(512, 544)          # hp cols (first needed ~12us)
            for j in range(1, NJ):
                load_ht_group(j)
            warmup(6)
            build_masks()
            build_pads()

            pass_j0_streamed()
            for i in range(4):
                build_chunk(i)
            for j in range(1, NJ - 1):
                # K passes first: they complete ktp[j-1]'s tail halo
                pass_m(j, 2)
                pass_m(j, 3)
                pass_m(j, 4)
                for i in range(4 * j, 4 * j + 4):
                    build_chunk(i)
                build_sliver(4 * j)
                b0 = 4 * (j - 1)
                for i in range(b0, b0 + 4):
                    block_scores(i)
                    block_combine(i)
                pass_m(j, 0)
                pass_m(j, 1)
            # final iteration: weave the last EIGHT blocks against all five
            # of group 3's projection passes so their mask/exp latencies and
            # the tail store drain inside the matmul stream
            pass_m(3, 2)
            pass_m(3, 3)
            pass_m(3, 4)
            block_scores(8)
            block_scores(9)
            build_sliver(12)
            pass_m(3, 0)
            block_combine(8)
            block_combine(9)
            block_scores(10)
            block_scores(11)
            pass_m(3, 1)
            block_combine(10)
            block_combine(11)
            for i in range(12, 16):
                build_chunk(i)
            build_sliver(16)
            block_scores(12)
            block_scores(13)
            block_combine(12)
            block_combine(13)
            block_scores(14)
            block_scores(15)
            block_combine(14)
            block_combine(15)

    nc.compile()
    return nc


_NC_CACHE = {}


def _get_nc():
    if "nc" not in _NC_CACHE:
        _NC_CACHE["nc"] = build_nc()
    return _NC_CACHE["nc"]


def kernel(h_base, tau, W_Q, W_K, W_proj):
    h_base = np.asarray(h_base, dtype=np.float32)
    tau_f = float(np.asarray(tau))
    wcat = np.concatenate(
        [np.asarray(W_Q, np.float32) / SCALE,
         np.asarray(W_K, np.float32),
         np.asarray(W_proj, np.float32) / tau_f], axis=1
    ).astype(ml_dtypes.bfloat16)

    nc = _get_nc()
    in_maps = [
        {"ht": np.ascontiguousarray(
            h_base[b].T.astype(ml_dtypes.bfloat16)), "wcat": wcat}
        for b in range(B)
    ]
    res = run_bass_kernel_spmd(nc, in_maps, list(range(N_CORES)))
    return np.stack([postprocess(np.asarray(res.results[b]["out"]))
                     for b in range(B)])


def postprocess(raw):
    # raw: [T, KL+1] = [est @ hp_win | band denom]; logits = raw[:,:KL]/denom
    logits = raw[:, :KL].astype(np.float32) / raw[:, KL:KL + 1]
    logits -= logits.max(axis=-1, keepdims=True)
    e = np.exp(logits)
    return (e / e.sum(axis=-1, keepdims=True)).astype(np.float32)
